# revision 2
# baseline (speedup 1.0000x reference)
"""Trainium2 Bass kernel for linear multi-head attention (Efficient Attention).

Reference computation (B=4, N=4096, D=1024, H=16, DK=64):
    q = softmax(x_q @ Wq.T + bq, axis=-1) / sqrt(DK)   (per-head, over DK)
    k = softmax(x_k @ Wk.T + bk, axis=-2)              (over sequence)
    v = x_v @ Wv.T + bv
    ctx = k^T v per head; out = (q @ ctx) @ Wo.T + bo

Sharding: 8 cores = (batch b, sequence half s); each core owns 2048 tokens of
one batch. k-softmax and ctx need full-sequence sums -> tiny paired AllReduce
of per-head [C|E] partials (C = exp(k)^T v, E = sum_n exp(k)), everything else
is token-local. Softmax max-subtraction is skipped: |logits| <= ~4 here, and
softmax is shift-invariant, so exp() is safe and the ratio is unchanged.

Layout strategy (PE needs the contraction dim on partitions): the host
pre-transposes/pre-tiles x and W into DMA-friendly blocks, so the device never
transposes anything. Matmuls run in bf16 (1 cycle/row; fp32r measured 2
cycles/row on this silicon) with fp32 PSUM accumulation. Biases ride in as
K=1 rank-1 matmuls (K layout has the bias along the free dim); the Q bias is
fused into the exp() activation, which is per-partition in the Q^T layout.
"""

import os
import sys
import types

import numpy as np
import ml_dtypes

import concourse.bass as bass
import concourse.mybir as mybir
import concourse.tile as tile
from concourse import bacc
from concourse.bass_utils import run_bass_kernel_spmd

# bass_utils' trace path hard-imports antenv.axon_hooks, which this image
# lacks. Provide a stub (get -> None => tracing skipped, run still works);
# test.py replaces the getter with a real NTFF hook for profiling runs.
try:
    import antenv.axon_hooks  # noqa: F401
except ImportError:
    _hooks = types.ModuleType("antenv.axon_hooks")
    _hooks._hook = None

    def _set_hook(h):
        _hooks._hook = h

    _hooks.set_axon_ntff_profile_hook = _set_hook
    _hooks.get_axon_ntff_profile_hook = lambda: _hooks._hook
    sys.modules["antenv.axon_hooks"] = _hooks
    import antenv

    antenv.axon_hooks = _hooks

B, N, D, H = 4, 4096, 1024, 16
DK = D // H
NCORES = 8
T = (B * N) // NCORES  # 2048 tokens per core
TT = T // 128          # 16 token tiles
T4 = T // 512          # 4 wide token tiles
C8 = D // 128          # 8 contraction chunks
NPAIR = H // 2         # 8 head pairs (2 heads of 64 = 128 partitions)

f32 = mybir.dt.float32
bf16 = mybir.dt.bfloat16
BF = ml_dtypes.bfloat16

LAST_RESULTS = None  # BassKernelResults of the most recent run (for test.py)
_CACHED = None


def _build():
    nc = bacc.Bacc("TRN2", target_bir_lowering=False, debug=False,
                   num_devices=NCORES)

    xq = nc.dram_tensor("xq", [T4, 128, C8, 512], bf16, kind="ExternalInput").ap()
    xk = nc.dram_tensor("xk", [TT, 128, C8, 128], bf16, kind="ExternalInput").ap()
    xv = nc.dram_tensor("xv", [TT, 128, C8, 128], bf16, kind="ExternalInput").ap()
    wq = nc.dram_tensor("wq", [C8, 128, D], bf16, kind="ExternalInput").ap()
    wk = nc.dram_tensor("wk", [C8, 128, D], bf16, kind="ExternalInput").ap()
    wv = nc.dram_tensor("wv", [C8, 128, D], bf16, kind="ExternalInput").ap()
    wo = nc.dram_tensor("wo", [C8, 128, D], bf16, kind="ExternalInput").ap()
    bq = nc.dram_tensor("bq", [128, C8], f32, kind="ExternalInput").ap()
    # bk is dropped entirely: the k-softmax runs over the sequence axis and a
    # per-channel bias shifts every sequence element equally -> exp(bk) cancels.
    # bv folds into ctx post-collective (C+E*bv), bo is added on the host.
    bvt = nc.dram_tensor("bvt", [128, NPAIR, 128], bf16, kind="ExternalInput").ap()
    blkd = nc.dram_tensor("blkd", [128, 128], bf16, kind="ExternalInput").ap()
    out = nc.dram_tensor("out", [T, D], f32, kind="ExternalOutput").ap()

    with tile.TileContext(nc) as tc:
        with (
            tc.tile_pool(name="const", bufs=1) as const,
            tc.tile_pool(name="wqo", bufs=1) as wqo,
            tc.tile_pool(name="dram", bufs=1, space="DRAM") as dram,
            tc.tile_pool(name="post", bufs=1) as post,
        ):
            # ---- constants / weights resident for the whole kernel ----
            # (const DMAs are emitted inside phase A so the first x/w tiles
            # win the DMA queue)
            blockones = const.tile([128, 128], bf16)
            bq_sb = const.tile([128, C8], f32)
            bvt_sb = const.tile([128, NPAIR, 128], bf16)
            # C|E accumulator: [p=d of head pair, pair, 128 e cols + 1 E col]
            ce_sb = const.tile([128, NPAIR, 129], f32)
            nc.vector.memset(ce_sb[:], 0.0)

            wq_sb = wqo.tile([128, C8, D], bf16)
            wo_sb = wqo.tile([128, C8, D], bf16)

            xqp_cm = tc.tile_pool(name="xqp", bufs=4)
            xqp = xqp_cm.__enter__()
            qexpp_cm = tc.tile_pool(name="qexpp", bufs=C8 + 2)
            qexpp = qexpp_cm.__enter__()
            qps_cm = tc.tile_pool(name="qps", bufs=2, space="PSUM")
            qps = qps_cm.__enter__()
            qexp_tiles = {}

            def emit_qproj(t4):
                # Q^T projection + exp for one 512-token block (PSUM: qps only,
                # so it can overlap the tail of phase A)
                xq_t = xqp.tile([128, C8, 512], bf16, tag="xq_t",
                                name=f"xq_t{t4}")
                nc.sync.dma_start(xq_t[:], xq[t4])
                for d in range(C8):  # 128-wide dout tile == head pair d
                    dsl = slice(d * 128, (d + 1) * 128)
                    qp = qps.tile([128, 512], f32, tag="qp")
                    for c in range(C8):
                        nc.tensor.matmul(qp[:], wq_sb[:, c, dsl],
                                         xq_t[:, c, :],
                                         start=(c == 0), stop=(c == C8 - 1))
                    qexp_t = qexpp.tile([128, 512], bf16, tag="qexp_t",
                                        name=f"qexp_t{t4}_{d}")
                    nc.scalar.activation(qexp_t[:], qp[:],
                                         mybir.ActivationFunctionType.Exp,
                                         bias=bq_sb[:, d:d + 1])
                    qexp_tiles[(t4, d)] = qexp_t

            # ================= phase A: K/V proj + exp + C/E =================
            with (
                tc.tile_pool(name="wkv", bufs=1) as wkv,
                tc.tile_pool(name="xkv", bufs=6) as xkv,
                tc.tile_pool(name="kvsb", bufs=3) as kvsb,
                tc.tile_pool(name="kvps", bufs=2, space="PSUM") as kvps,
                tc.tile_pool(name="ceps", bufs=2, space="PSUM") as ceps,
            ):
                wk_sb = wkv.tile([128, C8, D], bf16)
                wv_sb = wkv.tile([128, C8, D], bf16)

                for t in range(TT):
                    xk_t = xkv.tile([128, C8, 128], bf16, tag="xk_t")
                    xv_t = xkv.tile([128, C8, 128], bf16, tag="xv_t")
                    if t == 0:
                        # chunk-granular first tiles so the first matmuls'
                        # inputs land first in the DMA queues
                        nc.sync.dma_start(xk_t[:, 0:2, :], xk[t][:, 0:2, :])
                        nc.sync.dma_start(wk_sb[:, 0, :], wk[0])
                        nc.sync.dma_start(xk_t[:, 2:, :], xk[t][:, 2:, :])
                        nc.sync.dma_start(wk_sb[:, 1, :], wk[1])
                        nc.sync.dma_start(xv_t[:, 0:2, :], xv[t][:, 0:2, :])
                        nc.sync.dma_start(wv_sb[:, 0, :], wv[0])
                        nc.sync.dma_start(xv_t[:, 2:, :], xv[t][:, 2:, :])
                        nc.sync.dma_start(wv_sb[:, 1, :], wv[1])
                        for c in range(2, C8):
                            nc.sync.dma_start(wk_sb[:, c, :], wk[c])
                            nc.sync.dma_start(wv_sb[:, c, :], wv[c])
                    else:
                        nc.sync.dma_start(xk_t[:], xk[t])
                        nc.sync.dma_start(xv_t[:], xv[t])
                    if t == 1:
                        nc.sync.dma_start(blockones[:], blkd[:])
                        nc.sync.dma_start(bq_sb[:], bq[:])
                        nc.sync.dma_start(bvt_sb[:], bvt[:])
                    elif 2 <= t <= C8 + 1:
                        # spread the phase-C weight prefetch across phase A
                        nc.sync.dma_start(wq_sb[:, t - 2, :], wq[t - 2])
                        nc.sync.dma_start(wo_sb[:, t - 2, :], wo[t - 2])

                    kexp_t = kvsb.tile([128, D], bf16, tag="kexp_t")
                    # v with a ones column appended per pair for the E sums
                    v_t = kvsb.tile([128, NPAIR, 129], bf16, tag="v_t")
                    nc.vector.memset(v_t[:, :, 128], 1.0)

                    for d in range(2):  # dout halves of 512
                        ksl = slice(d * 512, (d + 1) * 512)
                        kps = kvps.tile([128, 512], f32, tag="kps")
                        for c in range(C8):
                            nc.tensor.matmul(kps[:], xk_t[:, c, :],
                                             wk_sb[:, c, ksl],
                                             start=(c == 0), stop=(c == C8 - 1))
                        nc.scalar.activation(kexp_t[:, ksl], kps[:],
                                             mybir.ActivationFunctionType.Exp)

                        vps = kvps.tile([128, 512], f32, tag="vps")
                        for c in range(C8):
                            nc.tensor.matmul(vps[:], xv_t[:, c, :],
                                             wv_sb[:, c, ksl],
                                             start=(c == 0), stop=(c == C8 - 1))
                        nc.vector.tensor_copy(
                            v_t[:, 4 * d:4 * d + 4, 0:128],
                            vps[:].rearrange("p (g e) -> p g e", g=4))

                    for p in range(NPAIR):
                        psl = slice(p * 128, (p + 1) * 128)
                        cps = ceps.tile([128, 129], f32, tag="cps")
                        nc.tensor.matmul(cps[:], kexp_t[:, psl], v_t[:, p, :],
                                         start=True, stop=True)
                        nc.vector.tensor_add(ce_sb[:, p, :], ce_sb[:, p, :],
                                             cps[:])

                emit_qproj(0)

            # ============ paired AllReduce of C|E over sequence halves ============
            # gpsimd DMAs: keeps the Sync HWDGE queue free for xq prefetches
            cc_in = dram.tile([128, NPAIR * 129], f32)
            cc_out = dram.tile([128, NPAIR * 129], f32)
            nc.gpsimd.dma_start(cc_in[:], ce_sb[:].rearrange("p a b -> p (a b)"))
            nc.gpsimd.collective_compute(
                "AllReduce",
                mybir.AluOpType.add,
                replica_groups=[[0, 1], [2, 3], [4, 5], [6, 7]],
                ins=[cc_in.opt()],
                outs=[cc_out.opt()],
            )
            ce_r = post.tile([128, NPAIR, 129], f32)
            nc.gpsimd.dma_start(ce_r[:], cc_out[:].rearrange("p (a b) -> p a b",
                                                             a=NPAIR))
            rec_e = post.tile([128, NPAIR], f32)
            ctx_sb = post.tile([128, NPAIR, 128], bf16)

            def emit_ctx():
                # ctx = (C + E x bv) / (8E) = C*recE/8 + bvt (bvt = bv/8 on the
                # in-head quadrants, 0 elsewhere); then zero cross-head quadrants
                nc.vector.reciprocal_approx_fast(rec_e[:], ce_r[:, :, 128])
                nc.vector.tensor_scalar_mul(rec_e[:], rec_e[:], 0.125)
                for p in range(NPAIR):
                    nc.vector.scalar_tensor_tensor(
                        ctx_sb[:, p, :], ce_r[:, p, 0:128], rec_e[:, p:p + 1],
                        bvt_sb[:, p, :],
                        op0=mybir.AluOpType.mult, op1=mybir.AluOpType.add)
                for p in range(NPAIR):
                    nc.vector.tensor_scalar(ctx_sb[0:64, p, 64:128],
                                            ce_r[0:64, p, 64:128], 0.0, None,
                                            op0=mybir.AluOpType.mult)
                    nc.vector.tensor_scalar(ctx_sb[64:128, p, 0:64],
                                            ce_r[64:128, p, 0:64], 0.0, None,
                                            op0=mybir.AluOpType.mult)

            # ======== phase C/D/E: Q softmax + attention + out proj ========
            with (
                tc.tile_pool(name="qwork", bufs=3) as qwork,
                tc.tile_pool(name="qsoft", bufs=4 * C8 + 2) as qsoft,
                tc.tile_pool(name="apool", bufs=2) as apool,
                tc.tile_pool(name="outp", bufs=4) as outp,
                tc.tile_pool(name="zps", bufs=2, space="PSUM") as zps,
                tc.tile_pool(name="aps", bufs=2, space="PSUM") as aps,
                tc.tile_pool(name="ops", bufs=2, space="PSUM") as ops,
            ):
                qs_tiles = {}

                def emit_qsoftmax(t4):
                    for d in range(C8):
                        qexp_t = qexp_tiles.pop((t4, d))
                        zp = zps.tile([128, 512], f32, tag="zp")
                        nc.tensor.matmul(zp[:], blockones[:], qexp_t[:],
                                         start=True, stop=True)
                        rec_t = qwork.tile([128, 512], f32, tag="rec_t")
                        nc.vector.reciprocal_approx_fast(rec_t[:], zp[:])
                        qs_t = qsoft.tile([128, 512], bf16, tag="qs_t",
                                          name=f"qs_t{t4}_{d}")
                        nc.vector.tensor_mul(qs_t[:], qexp_t[:], rec_t[:])
                        qs_tiles[(t4, d)] = qs_t

                def emit_attn_out(t4):
                    a_t = apool.tile([128, C8, 512], bf16, tag="a_t",
                                     name=f"a_t{t4}")
                    for d in range(C8):
                        ap_ = aps.tile([128, 512], f32, tag="ap_")
                        nc.tensor.matmul(ap_[:], ctx_sb[:, d, :],
                                         qs_tiles.pop((t4, d))[:],
                                         start=True, stop=True)
                        nc.scalar.copy(a_t[:, d, :], ap_[:])
                    for tt in range(4):  # 128-token subtiles
                        tsl = slice(tt * 128, (tt + 1) * 128)
                        rows = slice(t4 * 512 + tt * 128,
                                     t4 * 512 + tt * 128 + 128)
                        for dh in range(2):
                            osl = slice(dh * 512, (dh + 1) * 512)
                            op_ = ops.tile([128, 512], f32, tag="op_")
                            for c in range(C8):
                                nc.tensor.matmul(op_[:], a_t[:, c, tsl],
                                                 wo_sb[:, c, osl],
                                                 start=(c == 0),
                                                 stop=(c == C8 - 1))
                            out_t = outp.tile([128, 512], f32, tag="out_t")
                            nc.scalar.copy(out_t[:], op_[:])
                            nc.sync.dma_start(out[rows, osl], out_t[:])

                emit_qsoftmax(0)
                for t4 in range(1, T4):
                    emit_qproj(t4)
                    emit_qsoftmax(t4)
                emit_ctx()
                for t4 in range(T4):
                    emit_attn_out(t4)

            qps_cm.__exit__(None, None, None)
            qexpp_cm.__exit__(None, None, None)
            xqp_cm.__exit__(None, None, None)

    nc.compile()
    return nc


def _block_ones():
    blk = np.zeros((128, 128), np.float32)
    blk[:64, :64] = 1.0
    blk[64:, 64:] = 1.0
    return blk.astype(BF)


def _bv_tile(bv):
    # bvt[d, p, e] = bv[p*128+e]/8 on in-head quadrants, 0 on cross-head ones
    bvt = np.broadcast_to(bv.reshape(NPAIR, 128) * 0.125,
                          (128, NPAIR, 128)).copy()
    bvt[:64, :, 64:] = 0.0
    bvt[64:, :, :64] = 0.0
    return bvt.astype(BF)


def _prep_core_inputs(Xq, Xk, Xv, weights):
    """Per-core input dict from this core's [T, D] bf16 slices."""
    m = dict(weights)
    m["xq"] = np.ascontiguousarray(
        Xq.reshape(T4, 512, C8, 128).transpose(0, 3, 2, 1))
    m["xk"] = np.ascontiguousarray(
        Xk.reshape(TT, 128, C8, 128).transpose(0, 3, 2, 1))
    m["xv"] = np.ascontiguousarray(
        Xv.reshape(TT, 128, C8, 128).transpose(0, 3, 2, 1))
    return m


def kernel(query, key, value, Wq, bq, Wk, bk, Wv, bv, Wo, bo):
    global LAST_RESULTS, _CACHED
    if _CACHED is None:
        _CACHED = _build()
    nc = _CACHED

    f = np.float32
    weights = {
        "wq": np.ascontiguousarray(np.asarray(Wq, f).T).astype(BF).reshape(C8, 128, D),
        "wk": np.ascontiguousarray(np.asarray(Wk, f).T).astype(BF).reshape(C8, 128, D),
        "wv": np.ascontiguousarray(np.asarray(Wv, f).T).astype(BF).reshape(C8, 128, D),
        "wo": np.ascontiguousarray(np.asarray(Wo, f).T).astype(BF).reshape(C8, 128, D),
        "bq": np.ascontiguousarray(np.asarray(bq, f).reshape(C8, 128).T),
        "bvt": _bv_tile(np.asarray(bv, f)),
        "blkd": _block_ones(),
    }
    query = np.asarray(query, f).astype(BF)
    key = np.asarray(key, f).astype(BF)
    value = np.asarray(value, f).astype(BF)

    in_maps = []
    for core in range(NCORES):
        b, s = divmod(core, 2)
        rows = slice(s * T, (s + 1) * T)
        in_maps.append(_prep_core_inputs(
            query[b, rows], key[b, rows], value[b, rows], weights))

    LAST_RESULTS = run_bass_kernel_spmd(
        nc, in_maps, core_ids=list(range(NCORES)),
        trace=bool(os.environ.get("BASS_TRACE")))

    full = np.empty((B, N, D), np.float32)
    for core in range(NCORES):
        b, s = divmod(core, 2)
        full[b, s * T:(s + 1) * T, :] = LAST_RESULTS.results[core]["out"]
    full += np.asarray(bo, f)  # output bias applied on host
    return full



# revision 10
# speedup vs baseline: 1.3594x; 1.3594x over previous
"""Trainium2 Bass kernel for linear multi-head attention (Efficient Attention).

Reference computation (B=4, N=4096, D=1024, H=16, DK=64):
    q = softmax(x_q @ Wq.T + bq, axis=-1) / sqrt(DK)   (per-head, over DK)
    k = softmax(x_k @ Wk.T + bk, axis=-2)              (over sequence)
    v = x_v @ Wv.T + bv
    ctx = k^T v per head; out = (q @ ctx) @ Wo.T + bo

Sharding: 8 cores = (batch b, sequence half s); each core owns 2048 tokens of
one batch. k-softmax and ctx need full-sequence sums -> tiny paired AllReduce
of per-head [C|E] partials (C = exp(k)^T v, E = sum_n exp(k)), everything else
is token-local. Softmax max-subtraction is skipped: |logits| <= ~4 here, and
softmax is shift-invariant, so exp() is safe and the ratio is unchanged.

The four [2048,1024]x[1024,1024] GEMMs run in fp8e4m3 with DoubleRow perf
mode (2 MACs/cell/cycle, contraction 256 per matmul). Inputs and weights are
quantized host-side with per-tensor power-of-2 scales (exactly compensated:
the Q/K exp() activations fold 1/S into their scale operand, the V scale
rides in the E-column value so ctx normalization cancels it, and the output
descale happens on the host). The softmax/ctx/attention intermediates stay
bf16; PSUM accumulation is fp32 throughout, so the only precision loss vs
the bf16 kernel is the fp8 input rounding (~6e-3 max-rel on the final
output, tolerance 2e-2). Biases: bk cancels in the sequence softmax, bv is
folded into ctx post-collective, bq rides the exp activation, bo on host.
"""

import os
import sys
import types

import numpy as np
import ml_dtypes

import concourse.bass as bass
import concourse.mybir as mybir
import concourse.tile as tile
from concourse import bacc
from concourse.bass_utils import run_bass_kernel_spmd

# bass_utils' trace path hard-imports antenv.axon_hooks, which this image
# lacks. Provide a stub (get -> None => tracing skipped, run still works);
# test.py replaces the getter with a real NTFF hook for profiling runs.
try:
    import antenv.axon_hooks  # noqa: F401
except ImportError:
    _hooks = types.ModuleType("antenv.axon_hooks")
    _hooks._hook = None

    def _set_hook(h):
        _hooks._hook = h

    _hooks.set_axon_ntff_profile_hook = _set_hook
    _hooks.get_axon_ntff_profile_hook = lambda: _hooks._hook
    sys.modules["antenv.axon_hooks"] = _hooks
    import antenv

    antenv.axon_hooks = _hooks

B, N, D, H = 4, 4096, 1024, 16
DK = D // H
NCORES = 8
T = (B * N) // NCORES  # 2048 tokens per core
TT = T // 128          # 16 token tiles
T4 = T // 512          # 4 wide token tiles
C8 = D // 128          # 8 contraction chunks
C4 = C8 // 2           # 4 DoubleRow chunk pairs (256 contraction each)
NPAIR = H // 2         # 8 head pairs (2 heads of 64 = 128 partitions)
SA = 8192.0            # fixed fp8 scale for the attention output a_t

f32 = mybir.dt.float32
bf16 = mybir.dt.bfloat16
fp8 = mybir.dt.float8e4
BF = ml_dtypes.bfloat16
F8 = ml_dtypes.float8_e4m3
DR = mybir.MatmulPerfMode.DoubleRow

LAST_RESULTS = None  # BassKernelResults of the most recent run (for test.py)
_CACHED = {}


def _build(sq, sk, sv):
    """Compile for given per-GEMM scales (power-of-2, baked as immediates).

    sq/sk/sv = Sx*Sw of the q/k/v projections; their PSUM results are
    scaled by these, compensated at exp()/ctx time.
    """
    nc = bacc.Bacc("TRN2", target_bir_lowering=False, debug=False,
                   num_devices=NCORES)

    xq = nc.dram_tensor("xq", [T4, 128, C8, 512], fp8, kind="ExternalInput").ap()
    xk = nc.dram_tensor("xk", [TT, 128, C8, 128], fp8, kind="ExternalInput").ap()
    xv = nc.dram_tensor("xv", [TT, 128, C8, 128], fp8, kind="ExternalInput").ap()
    wq = nc.dram_tensor("wq", [C8, 128, D], fp8, kind="ExternalInput").ap()
    wk = nc.dram_tensor("wk", [C8, 128, D], fp8, kind="ExternalInput").ap()
    wv = nc.dram_tensor("wv", [C8, 128, D], fp8, kind="ExternalInput").ap()
    wo = nc.dram_tensor("wo", [C8, 128, D], fp8, kind="ExternalInput").ap()
    bq = nc.dram_tensor("bq", [128, C8], f32, kind="ExternalInput").ap()
    # bk is dropped entirely: the k-softmax runs over the sequence axis and a
    # per-channel bias shifts every sequence element equally -> exp(bk) cancels.
    # bv folds into ctx post-collective (C+E*bv), bo is added on the host.
    bvt = nc.dram_tensor("bvt", [128, NPAIR, 128], bf16, kind="ExternalInput").ap()
    blkd = nc.dram_tensor("blkd", [128, 128], bf16, kind="ExternalInput").ap()
    out = nc.dram_tensor("out", [T, D], bf16, kind="ExternalOutput").ap()

    with tile.TileContext(nc) as tc:
        with (
            tc.tile_pool(name="const", bufs=1) as const,
            tc.tile_pool(name="wqo", bufs=1) as wqo,
            tc.tile_pool(name="dram", bufs=1, space="DRAM") as dram,
            tc.tile_pool(name="post", bufs=1) as post,
        ):
            # ---- constants / weights resident for the whole kernel ----
            # (const DMAs are emitted inside phase A so the first x/w tiles
            # win the DMA queue)
            blockones = const.tile([128, 128], bf16)
            bq_sb = const.tile([128, C8], f32)
            bvt_sb = const.tile([128, NPAIR, 128], bf16)
            # C|E accumulator: [p=d of head pair, pair, 128 e cols + 1 E col]
            ce_sb = const.tile([128, NPAIR, 129], f32)
            nc.vector.memset(ce_sb[:], 0.0)

            wq_sb = wqo.tile([128, C8, D], fp8)
            wo_sb = wqo.tile([128, C8, D], fp8)

            # long-lived pools, entered in stack order (popped LIFO at the end)
            xqp_cm = tc.tile_pool(name="xqp", bufs=4)
            xqp = xqp_cm.__enter__()
            qexpp_cm = tc.tile_pool(name="qexpp", bufs=C8 + 2)
            qexpp = qexpp_cm.__enter__()
            qwork_cm = tc.tile_pool(name="qwork", bufs=3)
            qwork = qwork_cm.__enter__()
            qsoft_cm = tc.tile_pool(name="qsoft", bufs=4 * C8 + 2)
            qsoft = qsoft_cm.__enter__()
            qps_cm = tc.tile_pool(name="qps", bufs=2, space="PSUM")
            qps = qps_cm.__enter__()
            qexp_tiles = {}

            def emit_qproj(t4):
                # Q^T projection + exp for one 512-token block (PSUM: qps only,
                # so it can overlap the tail of phase A)
                xq_t = xqp.tile([128, C8, 512], fp8, tag="xq_t",
                                name=f"xq_t{t4}")
                nc.sync.dma_start(xq_t[:], xq[t4])
                for d in range(C8):  # 128-wide dout tile == head pair d
                    dsl = slice(d * 128, (d + 1) * 128)
                    qp = qps.tile([128, 512], f32, tag="qp")
                    for c in range(C4):
                        nc.tensor.matmul(qp[:], wq_sb[:, 2 * c:2 * c + 2, dsl],
                                         xq_t[:, 2 * c:2 * c + 2, :],
                                         start=(c == 0), stop=(c == C4 - 1),
                                         perf_mode=DR)
                    qexp_t = qexpp.tile([128, 512], bf16, tag="qexp_t",
                                        name=f"qexp_t{t4}_{d}")
                    nc.scalar.activation(qexp_t[:], qp[:],
                                         mybir.ActivationFunctionType.Exp,
                                         bias=bq_sb[:, d:d + 1],
                                         scale=float(1.0 / sq))
                    qexp_tiles[(t4, d)] = qexp_t

            # ================= phase A: K/V proj + exp + C/E =================
            with (
                tc.tile_pool(name="wkv", bufs=1) as wkv,
                tc.tile_pool(name="xkv", bufs=6) as xkv,
                tc.tile_pool(name="kvsb", bufs=3) as kvsb,
                tc.tile_pool(name="kvps", bufs=1, space="PSUM") as kvps,
                tc.tile_pool(name="ceps", bufs=2, space="PSUM") as ceps,
            ):
                wk_sb = wkv.tile([128, C8, D], fp8)
                wv_sb = wkv.tile([128, C8, D], fp8)

                for t in range(TT):
                    xk_t = xkv.tile([128, C8, 128], fp8, tag="xk_t")
                    xv_t = xkv.tile([128, C8, 128], fp8, tag="xv_t")
                    if t == 0:
                        # pair-granular first tiles so the first matmuls'
                        # inputs land first in the DMA queues
                        nc.sync.dma_start(xk_t[:, 0:2, :], xk[t][:, 0:2, :])
                        nc.sync.dma_start(wk_sb[:, 0, :], wk[0])
                        nc.sync.dma_start(wk_sb[:, 1, :], wk[1])
                        nc.sync.dma_start(xk_t[:, 2:, :], xk[t][:, 2:, :])
                        nc.sync.dma_start(xv_t[:, 0:2, :], xv[t][:, 0:2, :])
                        nc.sync.dma_start(wv_sb[:, 0, :], wv[0])
                        nc.sync.dma_start(wv_sb[:, 1, :], wv[1])
                        nc.sync.dma_start(xv_t[:, 2:, :], xv[t][:, 2:, :])
                        for c in range(2, C8):
                            nc.sync.dma_start(wk_sb[:, c, :], wk[c])
                            nc.sync.dma_start(wv_sb[:, c, :], wv[c])
                    else:
                        nc.sync.dma_start(xk_t[:], xk[t])
                        nc.sync.dma_start(xv_t[:], xv[t])
                    if t == 1:
                        nc.sync.dma_start(blockones[:], blkd[:])
                        nc.sync.dma_start(bq_sb[:], bq[:])
                        nc.sync.dma_start(bvt_sb[:], bvt[:])
                    elif 2 <= t <= C8 + 1:
                        # spread the phase-C weight prefetch across phase A
                        nc.sync.dma_start(wq_sb[:, t - 2, :], wq[t - 2])
                        nc.sync.dma_start(wo_sb[:, t - 2, :], wo[t - 2])

                    kexp_t = kvsb.tile([128, D], bf16, tag="kexp_t")
                    # v with a ones column appended per pair for the E sums;
                    # the column value 8*sv makes 1/E' absorb both the ctx /8
                    # and the fp8 v-scale (ctx = C'/(8*sv*E) + bv/8).
                    v_t = kvsb.tile([128, NPAIR, 129], bf16, tag="v_t")
                    nc.vector.memset(v_t[:, :, 128], float(8.0 * sv))

                    # interleaved K/V DoubleRow accumulation, 2 output halves
                    # each: the stationary x-chunk-pair serves 2 matmuls
                    kps0 = kvps.tile([128, 512], f32, tag="kps0")
                    kps1 = kvps.tile([128, 512], f32, tag="kps1")
                    vps0 = kvps.tile([128, 512], f32, tag="vps0")
                    vps1 = kvps.tile([128, 512], f32, tag="vps1")
                    for c in range(C4):
                        cs = slice(2 * c, 2 * c + 2)
                        fl = dict(start=(c == 0), stop=(c == C4 - 1),
                                  perf_mode=DR)
                        nc.tensor.matmul(kps0[:], xk_t[:, cs, :],
                                         wk_sb[:, cs, 0:512], **fl)
                        nc.tensor.matmul(kps1[:], xk_t[:, cs, :],
                                         wk_sb[:, cs, 512:1024], **fl)
                        nc.tensor.matmul(vps0[:], xv_t[:, cs, :],
                                         wv_sb[:, cs, 0:512], **fl)
                        nc.tensor.matmul(vps1[:], xv_t[:, cs, :],
                                         wv_sb[:, cs, 512:1024], **fl)
                    nc.scalar.activation(kexp_t[:, 0:512], kps0[:],
                                         mybir.ActivationFunctionType.Exp,
                                         scale=float(1.0 / sk))
                    nc.scalar.activation(kexp_t[:, 512:1024], kps1[:],
                                         mybir.ActivationFunctionType.Exp,
                                         scale=float(1.0 / sk))
                    nc.vector.tensor_copy(
                        v_t[:, 0:4, 0:128],
                        vps0[:].rearrange("p (g e) -> p g e", g=4))
                    nc.vector.tensor_copy(
                        v_t[:, 4:8, 0:128],
                        vps1[:].rearrange("p (g e) -> p g e", g=4))

                    for p in range(NPAIR):
                        psl = slice(p * 128, (p + 1) * 128)
                        cps = ceps.tile([128, 129], f32, tag="cps")
                        nc.tensor.matmul(cps[:], kexp_t[:, psl], v_t[:, p, :],
                                         start=True, stop=True)
                        nc.vector.tensor_add(ce_sb[:, p, :], ce_sb[:, p, :],
                                             cps[:])

                emit_qproj(0)

            # ============ paired AllReduce of C|E over sequence halves ============
            # gpsimd DMAs: keeps the Sync HWDGE queue free for xq prefetches
            cc_in = dram.tile([128, NPAIR * 129], f32)
            cc_out = dram.tile([128, NPAIR * 129], f32)
            nc.gpsimd.dma_start(cc_in[:], ce_sb[:].rearrange("p a b -> p (a b)"))
            nc.gpsimd.collective_compute(
                "AllReduce",
                mybir.AluOpType.add,
                replica_groups=[[0, 1], [2, 3], [4, 5], [6, 7]],
                ins=[cc_in.opt()],
                outs=[cc_out.opt()],
            )
            ce_r = post.tile([128, NPAIR, 129], f32)
            nc.gpsimd.dma_start(ce_r[:], cc_out[:].rearrange("p (a b) -> p a b",
                                                             a=NPAIR))
            rec_e = post.tile([128, NPAIR], f32)
            ctx_sb = post.tile([128, NPAIR, 128], bf16)

            def emit_ctx():
                # ctx = (C + E x bv) / (8E) = C'*recE' + bvt (bvt = bv/8 on the
                # in-head quadrants, 0 elsewhere; E' = 8*sv*E absorbs the v
                # scale); then zero cross-head quadrants
                nc.vector.reciprocal_approx_fast(rec_e[:], ce_r[:, :, 128])
                for p in range(NPAIR):
                    nc.vector.scalar_tensor_tensor(
                        ctx_sb[:, p, :], ce_r[:, p, 0:128], rec_e[:, p:p + 1],
                        bvt_sb[:, p, :],
                        op0=mybir.AluOpType.mult, op1=mybir.AluOpType.add)
                for p in range(NPAIR):
                    nc.vector.tensor_scalar(ctx_sb[0:64, p, 64:128],
                                            ce_r[0:64, p, 64:128], 0.0, None,
                                            op0=mybir.AluOpType.mult)
                    nc.vector.tensor_scalar(ctx_sb[64:128, p, 0:64],
                                            ce_r[64:128, p, 0:64], 0.0, None,
                                            op0=mybir.AluOpType.mult)

            # ======== phase C: Q softmax (zp via blockones matmul) ========
            zps_cm = tc.tile_pool(name="zps", bufs=2, space="PSUM")
            zps = zps_cm.__enter__()
            qs_tiles = {}

            def emit_qsoftmax(t4):
                for d in range(C8):
                    qexp_t = qexp_tiles.pop((t4, d))
                    zp = zps.tile([128, 512], f32, tag="zp")
                    nc.tensor.matmul(zp[:], blockones[:], qexp_t[:],
                                     start=True, stop=True)
                    rec_t = qwork.tile([128, 512], f32, tag="rec_t")
                    nc.vector.reciprocal_approx_fast(rec_t[:], zp[:])
                    qs_t = qsoft.tile([128, 512], bf16, tag="qs_t",
                                      name=f"qs_t{t4}_{d}")
                    nc.vector.tensor_mul(qs_t[:], qexp_t[:], rec_t[:])
                    qs_tiles[(t4, d)] = qs_t

            emit_qsoftmax(0)
            for t4 in range(1, T4):
                emit_qproj(t4)
                emit_qsoftmax(t4)

            # free the Q-projection PSUM pools before opening the attn ones
            zps_cm.__exit__(None, None, None)
            qps_cm.__exit__(None, None, None)

            # ======== phase D/E: attention + out proj (DoubleRow fp8) ========
            with (
                tc.tile_pool(name="apool", bufs=2) as apool,
                tc.tile_pool(name="outp", bufs=4) as outp,
                tc.tile_pool(name="aps", bufs=2, space="PSUM") as aps,
                tc.tile_pool(name="ops", bufs=2, space="PSUM") as ops,
            ):
                def emit_attn_out(t4):
                    a_t = apool.tile([128, C8, 512], fp8, tag="a_t",
                                     name=f"a_t{t4}")
                    for d in range(C8):
                        ap_ = aps.tile([128, 512], f32, tag="ap_")
                        nc.tensor.matmul(ap_[:], ctx_sb[:, d, :],
                                         qs_tiles.pop((t4, d))[:],
                                         start=True, stop=True)
                        nc.vector.tensor_scalar(a_t[:, d, :], ap_[:], SA, None,
                                                op0=mybir.AluOpType.mult)
                    for tt in range(4):  # 128-token subtiles
                        tsl = slice(tt * 128, (tt + 1) * 128)
                        rows = slice(t4 * 512 + tt * 128,
                                     t4 * 512 + tt * 128 + 128)
                        op0 = ops.tile([128, 512], f32, tag="op0")
                        op1 = ops.tile([128, 512], f32, tag="op1")
                        for c in range(C4):
                            cs = slice(2 * c, 2 * c + 2)
                            fl = dict(start=(c == 0), stop=(c == C4 - 1),
                                      perf_mode=DR)
                            # stationary a_t chunk-pair serves both halves
                            nc.tensor.matmul(op0[:], a_t[:, cs, tsl],
                                             wo_sb[:, cs, 0:512], **fl)
                            nc.tensor.matmul(op1[:], a_t[:, cs, tsl],
                                             wo_sb[:, cs, 512:1024], **fl)
                        for dh, op_ in ((0, op0), (1, op1)):
                            osl = slice(dh * 512, (dh + 1) * 512)
                            out_t = outp.tile([128, 512], bf16, tag="out_t")
                            nc.vector.tensor_copy(out_t[:], op_[:])
                            nc.sync.dma_start(out[rows, osl], out_t[:])

                emit_ctx()
                for t4 in range(T4):
                    emit_attn_out(t4)

            qsoft_cm.__exit__(None, None, None)
            qwork_cm.__exit__(None, None, None)
            qexpp_cm.__exit__(None, None, None)
            xqp_cm.__exit__(None, None, None)

    nc.compile()
    return nc


def _block_ones():
    blk = np.zeros((128, 128), np.float32)
    blk[:64, :64] = 1.0
    blk[64:, 64:] = 1.0
    return blk.astype(BF)


def _bv_tile(bv):
    # bvt[d, p, e] = bv[p*128+e]/8 on in-head quadrants, 0 on cross-head ones
    bvt = np.broadcast_to(bv.reshape(NPAIR, 128) * 0.125,
                          (128, NPAIR, 128)).copy()
    bvt[:64, :, 64:] = 0.0
    bvt[64:, :, :64] = 0.0
    return bvt.astype(BF)


def _scale_pow2(a, target=112.0):
    return np.float32(2.0 ** np.floor(np.log2(target / np.abs(a).max())))


def _prep_core_inputs(Xq, Xk, Xv, weights):
    """Per-core input dict from this core's [T, D] fp8 slices."""
    m = dict(weights)
    m["xq"] = np.ascontiguousarray(
        Xq.reshape(T4, 512, C8, 128).transpose(0, 3, 2, 1))
    m["xk"] = np.ascontiguousarray(
        Xk.reshape(TT, 128, C8, 128).transpose(0, 3, 2, 1))
    m["xv"] = np.ascontiguousarray(
        Xv.reshape(TT, 128, C8, 128).transpose(0, 3, 2, 1))
    return m


def kernel(query, key, value, Wq, bq, Wk, bk, Wv, bv, Wo, bo):
    global LAST_RESULTS, _CACHED

    f = np.float32
    query = np.asarray(query, f)
    key = np.asarray(key, f)
    value = np.asarray(value, f)
    WqT = np.ascontiguousarray(np.asarray(Wq, f).T)
    WkT = np.ascontiguousarray(np.asarray(Wk, f).T)
    WvT = np.ascontiguousarray(np.asarray(Wv, f).T)
    WoT = np.ascontiguousarray(np.asarray(Wo, f).T)

    sxq, sxk, sxv = _scale_pow2(query), _scale_pow2(key), _scale_pow2(value)
    swq, swk, swv, swo = (_scale_pow2(WqT), _scale_pow2(WkT),
                          _scale_pow2(WvT), _scale_pow2(WoT))
    scales = (float(sxq * swq), float(sxk * swk), float(sxv * swv))
    if scales not in _CACHED:
        _CACHED[scales] = _build(*scales)
    nc = _CACHED[scales]

    weights = {
        "wq": (WqT * swq).astype(F8).reshape(C8, 128, D),
        "wk": (WkT * swk).astype(F8).reshape(C8, 128, D),
        "wv": (WvT * swv).astype(F8).reshape(C8, 128, D),
        "wo": (WoT * swo).astype(F8).reshape(C8, 128, D),
        "bq": np.ascontiguousarray(np.asarray(bq, f).reshape(C8, 128).T),
        "bvt": _bv_tile(np.asarray(bv, f)),
        "blkd": _block_ones(),
    }
    q8 = (query * sxq).astype(F8)
    k8 = (key * sxk).astype(F8)
    v8 = (value * sxv).astype(F8)

    in_maps = []
    for core in range(NCORES):
        b, s = divmod(core, 2)
        rows = slice(s * T, (s + 1) * T)
        in_maps.append(_prep_core_inputs(
            q8[b, rows], k8[b, rows], v8[b, rows], weights))

    LAST_RESULTS = run_bass_kernel_spmd(
        nc, in_maps, core_ids=list(range(NCORES)),
        trace=bool(os.environ.get("BASS_TRACE")))

    descale = 1.0 / (SA * float(swo))
    full = np.empty((B, N, D), np.float32)
    for core in range(NCORES):
        b, s = divmod(core, 2)
        full[b, s * T:(s + 1) * T, :] = (
            LAST_RESULTS.results[core]["out"].astype(f) * descale)
    full += np.asarray(bo, f)  # output bias applied on host
    return full
